# revision 9
# baseline (speedup 1.0000x reference)
"""NonLocalBlock (nn_NonLocalBlock_80221399155245) — Trainium2 Bass kernel.

Sharding: data-parallel over batch B=8, one batch item per NeuronCore.
Per-core pipeline (xf = x[b] as [C=256, N=4096]):
  theta = Wq @ xf, phi = Wk @ xf        [I=128, N]  (bf16, I-major)
  gT    = (Wg @ xf).T                   (N-major 128-chunks, bf16)
  Main loop over nb=512 column blocks, software-pipelined 3 deep so the
  PE never idles (p-state ramp) while ACT exp paces the loop:
    seg kb: PE: logits(kb) interleaved with outT(kb-1), bc(kb-1),
                ypj(kb-2)
            ACT: exp(kb) groups (PSUM->SBUF bf16)
            DVE: pairwise tree colsum(kb-1) in bf16, ypj drain(kb-3)
                 + bn_stats, on(kb-1) = outT/bc divide
  BatchNorm batch-stats are AllReduced across the 8 cores (sync-BN
  exact), SE channel attention computed in the prologue, residual fused
  in the epilogue via ACT affine (scale/bias per channel) + DVE add.
"""

import numpy as np
import ml_dtypes
import concourse.bass as bass
import concourse.tile as tile
from concourse import bacc, mybir
from concourse.bass_utils import run_bass_kernel_spmd

F32 = mybir.dt.float32
BF16 = mybir.dt.bfloat16
F32R = mybir.dt.float32r
AF = mybir.ActivationFunctionType
ALU = mybir.AluOpType

C = 256     # channels
I = 128     # inter channels
R = 64      # SE reduction
P = 128     # SBUF partitions
B = 8       # batch == cores
H = W = 64
N = H * W   # 4096 pixels
NB = 512    # n-block columns
CHUNK_GROUP = 2   # logits chunks per exp-activation group


def _build(n_cores=B, nn=N, nb=NB, chunk_group=CHUNK_GROUP, total_pixels=None):
    M = nn // P
    NBLK = nn // nb
    GRP = M // chunk_group
    assert M % chunk_group == 0
    if total_pixels is None:
        total_pixels = n_cores * nn
    sm_scale = float(1.0 / np.sqrt(np.float32(I)))

    nc = bacc.Bacc("TRN2", target_bir_lowering=False, debug=False,
                   num_devices=n_cores)

    x_d = nc.declare_dram_parameter("x", [C, nn], F32, isOutput=False)
    wq_d = nc.declare_dram_parameter("wq_t", [C, I], BF16, isOutput=False)
    wk_d = nc.declare_dram_parameter("wk_t", [C, I], BF16, isOutput=False)
    wg_d = nc.declare_dram_parameter("wg_t", [C, I], BF16, isOutput=False)
    wo_d = nc.declare_dram_parameter("wo_t", [I, C], BF16, isOutput=False)
    fc1w_d = nc.declare_dram_parameter("fc1_wt", [C, R], F32, isOutput=False)
    fc1b_d = nc.declare_dram_parameter("fc1_b", [R], F32, isOutput=False)
    fc2w_d = nc.declare_dram_parameter("fc2_wt", [R, C], F32, isOutput=False)
    fc2bn_d = nc.declare_dram_parameter("fc2_bn", [C], F32, isOutput=False)
    gam_d = nc.declare_dram_parameter("bn_gamma", [C], F32, isOutput=False)
    bet_d = nc.declare_dram_parameter("bn_beta", [C], F32, isOutput=False)
    out_d = nc.declare_dram_parameter("out", [C, nn], F32, isOutput=True)

    bn_in = nc.dram_tensor("bn_in", [P, 4], F32)
    bn_out = nc.dram_tensor("bn_out", [P, 4], F32,
                            addr_space="Shared" if n_cores > 4 else "Local")

    with tile.TileContext(nc) as tc:
        import contextlib
        with contextlib.ExitStack() as stack:
            sing = stack.enter_context(tc.tile_pool(name="sing", bufs=1))

            xf32 = [sing.tile([P, nn], F32, tag=f"xf32_{cc}", name=f"xf32_{cc}")
                    for cc in range(2)]
            xbf = [sing.tile([P, nn], BF16, tag=f"xbf_{cc}", name=f"xbf_{cc}")
                   for cc in range(2)]
            theta = sing.tile([P, nn], BF16, tag="theta", name="theta")
            phi = sing.tile([P, nn], BF16, tag="phi", name="phi")
            gT = sing.tile([P, nn], BF16, tag="gT", name="gT")
            ysb_all = sing.tile([P, 2, nn], BF16, tag="ysb_all", name="ysb_all")
            bnst = [sing.tile([P, NBLK, 6], F32, tag=f"bnst_{cc}",
                              name=f"bnst_{cc}") for cc in range(2)]

            wq = sing.tile([P, 2, I], BF16, tag="wq", name="wq")
            wk = sing.tile([P, 2, I], BF16, tag="wk", name="wk")
            wg = sing.tile([P, 2, I], BF16, tag="wg", name="wg")
            wo = sing.tile([P, 2, P], BF16, tag="wo", name="wo")
            fc1w = sing.tile([P, 2, R], F32, tag="fc1w", name="fc1w")
            fc1b = sing.tile([R, 1], F32, tag="fc1b", name="fc1b")
            fc2w = sing.tile([R, 2, P], F32, tag="fc2w", name="fc2w")
            fc2bn = sing.tile([P, 2], F32, tag="fc2bn", name="fc2bn")
            gam = sing.tile([P, 2], F32, tag="gam", name="gam")
            bet = sing.tile([P, 2], F32, tag="bet", name="bet")
            ones128 = sing.tile([P, P], BF16, tag="ones128", name="ones128")
            chw = sing.tile([P, 2], F32, tag="chw", name="chw")
            pooled = sing.tile([P, 2], F32, tag="pooled", name="pooled")

            nc.vector.memset(ones128, 1.0)

            # weights first (tiny), then x in column-chunk pairs
            nc.sync.dma_start(out=wq, in_=wq_d.rearrange("(a p) i -> p a i", p=P))
            nc.sync.dma_start(out=wk, in_=wk_d.rearrange("(a p) i -> p a i", p=P))
            nc.sync.dma_start(out=wg, in_=wg_d.rearrange("(a p) i -> p a i", p=P))
            nc.sync.dma_start(out=wo, in_=wo_d.rearrange("i (a c) -> i a c", a=2))
            nc.sync.dma_start(out=fc1w, in_=fc1w_d.rearrange("(a p) r -> p a r", p=P))
            nc.sync.dma_start(out=fc1b, in_=fc1b_d[:, None])
            nc.sync.dma_start(out=fc2w, in_=fc2w_d.rearrange("r (a c) -> r a c", a=2))
            nc.sync.dma_start(out=fc2bn, in_=fc2bn_d.rearrange("(a p) -> p a", p=P))
            nc.sync.dma_start(out=gam, in_=gam_d.rearrange("(a p) -> p a", p=P))
            nc.sync.dma_start(out=bet, in_=bet_d.rearrange("(a p) -> p a", p=P))

            NXCH = 4            # x DMA column chunks
            xw = nn // NXCH
            for t in range(NXCH):
                sl = slice(t * xw, (t + 1) * xw)
                for cc in range(2):
                    nc.sync.dma_start(out=xf32[cc][:, sl],
                                      in_=x_d[cc * P:(cc + 1) * P, sl])
                for cc in range(2):
                    nc.vector.tensor_copy(xbf[cc][:, sl], xf32[cc][:, sl])

            # ---- prologue: QKV projections + SE ----
            NCH = nn // 512
            with tc.tile_pool(name="proj_ps", bufs=4, space="PSUM") as pps, \
                 tc.tile_pool(name="g_ps", bufs=2, space="PSUM") as gps, \
                 tc.tile_pool(name="se_ps", bufs=1, space="PSUM") as seps:
                # phi fully first (block 0 logits need all of phi), theta
                # chunk 0 early, then the rest
                proj_order = [(wk, phi, t) for t in range(NCH)]
                proj_order.insert(1, (wq, theta, 0))
                proj_order += [(wq, theta, t) for t in range(1, NCH)]
                for (wt, dst, t) in proj_order:
                    ps = pps.tile([P, 512], F32, tag="proj", name="proj")
                    for cc in range(2):
                        nc.tensor.matmul(
                            ps[:], wt[:, cc, :],
                            xbf[cc][:, t * 512:(t + 1) * 512],
                            start=(cc == 0), stop=(cc == 1))
                    nc.scalar.copy(dst[:, t * 512:(t + 1) * 512], ps[:])

                for q in range(M // 4):
                    ps2 = gps.tile([P, 4, I], F32, tag="gproj", name="gproj")
                    for j in range(4):
                        mj = q * 4 + j
                        for cc in range(2):
                            nc.tensor.matmul(
                                ps2[:, j, :], xbf[cc][:, mj * P:(mj + 1) * P],
                                wg[:, cc, :], start=(cc == 0), stop=(cc == 1))
                    nc.vector.tensor_copy(
                        gT[:, q * 4 * I:(q + 1) * 4 * I],
                        ps2[:, :, :].rearrange("p a b -> p (a b)"))

                # SE channel attention (prologue: PSUM free, loads Exp table
                # before the main loop needs it)
                for cc in range(2):
                    nc.vector.reduce_sum(pooled[:, cc:cc + 1], xf32[cc][:],
                                         axis=mybir.AxisListType.X)
                hps = seps.tile([R, 1], F32, tag="se", name="se_h")
                for cc in range(2):
                    nc.tensor.matmul(hps[:], fc1w[:, cc, :], pooled[:, cc:cc + 1],
                                     start=(cc == 0), stop=(cc == 1))
                hsb = sing.tile([R, 1], F32, tag="hsb", name="hsb")
                nc.scalar.activation(hsb[:], hps[:], AF.Relu, bias=fc1b[:])
                for cc in range(2):
                    zps = seps.tile([P, 1], F32, tag="se2", name="se_z")
                    nc.tensor.matmul(zps[:], fc2w[:, cc, :], hsb[:],
                                     start=True, stop=True)
                    esb = sing.tile([P, 1], F32, tag=f"esb_{cc}", name=f"esb_{cc}")
                    nc.scalar.activation(esb[:], zps[:], AF.Exp,
                                         bias=fc2bn[:, cc:cc + 1], scale=-1.0)
                    nc.vector.tensor_scalar_add(esb[:], esb[:], 1.0)
                    nc.vector.reciprocal(chw[:, cc:cc + 1], esb[:])

            # ---- main attention loop: software-pipelined 3 deep ----
            with tc.tile_pool(name="lg", bufs=2, space="PSUM") as lg, \
                 tc.tile_pool(name="aux_ps", bufs=1, space="PSUM") as aux, \
                 tc.tile_pool(name="pTp", bufs=2) as pTp, \
                 tc.tile_pool(name="smalls", bufs=2) as smalls:

                outT_ps = aux.tile([P, nb], F32, tag="outT", name="outT")
                bc_ps = aux.tile([P, nb], F32, tag="bc", name="bc")
                ypj_ps = aux.tile([P, 2, nb], F32, tag="ypj", name="ypj")

                pTs = [None] * NBLK
                ons = [None] * NBLK

                def emit_norm(kk):
                    # inv = 1/colsums (bc_ps holds broadcast colsums of
                    # block kk), on = outT * inv  — DVE
                    inv_sb = smalls.tile([P, nb], F32, tag="inv", name="inv")
                    nc.vector.reciprocal_approx_fast(out=inv_sb[:],
                                                     in_=bc_ps[:])
                    on_sb = smalls.tile([P, nb], BF16, tag="on", name="on")
                    ons[kk] = on_sb
                    nc.vector.tensor_tensor(on_sb[:], outT_ps[:], inv_sb[:],
                                            ALU.mult)

                for kb in range(NBLK + 3):
                    # --- DVE: normalize block kb-2 (deps done at end of
                    # segment kb-1); then drain(kb-3) ---
                    if 2 <= kb <= NBLK + 1:
                        emit_norm(kb - 2)
                    if 3 <= kb <= NBLK + 2:
                        k3 = kb - 3
                        nc.vector.tensor_scalar(
                            ysb_all[:, :, k3 * nb:(k3 + 1) * nb],
                            ypj_ps[:, :, :], 1.0, None, ALU.mult)
                        for cc in range(2):
                            nc.vector.bn_stats(
                                out=bnst[cc][:, k3, :],
                                in_=ysb_all[:, cc, k3 * nb:(k3 + 1) * nb])

                    # --- PE: logits(kb) interleaved with outT(kb-1) and
                    # colsum-broadcast cs(kb-1) ---
                    if kb < NBLK:
                        th_sl = theta[:, kb * nb:(kb + 1) * nb]
                        pT = pTp.tile([P, M * nb], BF16, tag="pT", name="pT")
                        pTs[kb] = pT
                        for g in range(GRP):
                            lgt = lg.tile([P, chunk_group, nb], F32,
                                          tag="lg", name="lg")
                            for j in range(chunk_group):
                                mj = g * chunk_group + j
                                nc.tensor.matmul(
                                    lgt[:, j, :], phi[:, mj * P:(mj + 1) * P],
                                    th_sl, start=True, stop=True)
                            nc.scalar.activation(
                                pT[:, g * chunk_group * nb:
                                   (g + 1) * chunk_group * nb],
                                lgt[:, :, :].rearrange("p a b -> p (a b)"),
                                AF.Exp, scale=sm_scale)
                            if kb >= 1:
                                for j in range(chunk_group):
                                    mj = g * chunk_group + j
                                    pslice = pTs[kb - 1][:, mj * nb:(mj + 1) * nb]
                                    nc.tensor.matmul(
                                        outT_ps[:],
                                        gT[:, mj * I:(mj + 1) * I], pslice,
                                        start=(mj == 0), stop=(mj == M - 1),
                                        skip_group_check=True)
                                    nc.tensor.matmul(
                                        bc_ps[:], ones128[:], pslice,
                                        start=(mj == 0), stop=(mj == M - 1),
                                        skip_group_check=True)
                    elif kb == NBLK:
                        # leftover outT + cs for the last block
                        for mj in range(M):
                            pslice = pTs[kb - 1][:, mj * nb:(mj + 1) * nb]
                            nc.tensor.matmul(
                                outT_ps[:], gT[:, mj * I:(mj + 1) * I], pslice,
                                start=(mj == 0), stop=(mj == M - 1),
                                skip_group_check=True)
                            nc.tensor.matmul(
                                bc_ps[:], ones128[:], pslice,
                                start=(mj == 0), stop=(mj == M - 1),
                                skip_group_check=True)

                    # --- PE: ypj(kb-2) = Wo @ on ---
                    if 2 <= kb <= NBLK + 1:
                        for cc in range(2):
                            nc.tensor.matmul(ypj_ps[:, cc, :], wo[:, cc, :],
                                             ons[kb - 2][:],
                                             start=True, stop=True)

            # ---- epilogue: sync-BN, affine, residual ----
            with tc.tile_pool(name="epi", bufs=2) as epi:
                stats = sing.tile([P, 4], F32, tag="stats", name="stats")
                for cc in range(2):
                    mv = epi.tile([P, 2], F32, tag="mv", name="mv")
                    nc.vector.bn_aggr(out=mv[:], in_=bnst[cc][:, :, :])
                    # sum = mean*nn ; sumsq = (var + mean^2)*nn
                    nc.vector.tensor_scalar_mul(stats[:, cc:cc + 1],
                                                mv[:, 0:1], float(nn))
                    m2 = epi.tile([P, 1], F32, tag="m2", name="m2")
                    nc.vector.tensor_tensor(m2[:], mv[:, 0:1], mv[:, 0:1],
                                            ALU.mult)
                    nc.vector.tensor_tensor(m2[:], mv[:, 1:2], m2[:], ALU.add)
                    nc.vector.tensor_scalar_mul(stats[:, 2 + cc:3 + cc],
                                                m2[:], float(nn))
                nc.sync.dma_start(out=bn_in[:], in_=stats[:])
                nc.gpsimd.collective_compute(
                    "AllReduce", ALU.add,
                    replica_groups=[list(range(n_cores))],
                    ins=[bn_in[:]], outs=[bn_out[:]])
                stats_g = sing.tile([P, 4], F32, tag="stats_g", name="stats_g")
                nc.gpsimd.dma_start(out=stats_g[:], in_=bn_out[:])

                inv_np = 1.0 / float(total_pixels)
                half = nn // 2
                for cc in range(2):
                    mean = epi.tile([P, 1], F32, tag="mean", name="mean")
                    ex2 = epi.tile([P, 1], F32, tag="ex2", name="ex2")
                    nc.vector.tensor_scalar_mul(mean[:], stats_g[:, cc:cc + 1],
                                                inv_np)
                    nc.vector.tensor_scalar_mul(ex2[:], stats_g[:, 2 + cc:3 + cc],
                                                inv_np)
                    var = epi.tile([P, 1], F32, tag="var", name="var")
                    nc.vector.tensor_tensor(var[:], mean[:], mean[:], ALU.mult)
                    nc.vector.tensor_tensor(var[:], ex2[:], var[:], ALU.subtract)
                    nc.vector.tensor_scalar_add(var[:], var[:], 1e-5)
                    rv = epi.tile([P, 1], F32, tag="rv", name="rv")
                    nc.vector.reciprocal(rv[:], var[:])
                    istd = epi.tile([P, 1], F32, tag="istd", name="istd")
                    nc.scalar.activation(istd[:], rv[:], AF.Sqrt)
                    g1 = epi.tile([P, 1], F32, tag="g1", name="g1")
                    nc.vector.tensor_tensor(g1[:], istd[:], gam[:, cc:cc + 1],
                                            ALU.mult)
                    A = epi.tile([P, 1], F32, tag="A", name="A")
                    nc.vector.tensor_tensor(A[:], g1[:], chw[:, cc:cc + 1],
                                            ALU.mult)
                    Bt = epi.tile([P, 1], F32, tag="Bt", name="Bt")
                    nc.vector.tensor_tensor(Bt[:], mean[:], g1[:], ALU.mult)
                    nc.vector.tensor_tensor(Bt[:], bet[:, cc:cc + 1], Bt[:],
                                            ALU.subtract)
                    nc.vector.tensor_tensor(Bt[:], Bt[:], chw[:, cc:cc + 1],
                                            ALU.mult)

                    for h in range(4):
                        qw = nn // 4
                        sl = slice(h * qw, (h + 1) * qw)
                        tf = epi.tile([P, qw], F32, tag="tf", name="tf")
                        nc.scalar.activation(tf[:], ysb_all[:, cc, sl],
                                             AF.Identity, bias=Bt[:],
                                             scale=A[:])
                        osb = epi.tile([P, qw], F32, tag="osb", name="osb")
                        nc.vector.tensor_tensor(osb[:], tf[:],
                                                xf32[cc][:, sl], ALU.add)
                        nc.sync.dma_start(out=out_d[cc * P:(cc + 1) * P, sl],
                                          in_=osb[:])

    nc.compile()
    return nc


_NC_CACHE = {}


def _get_nc(**kw):
    key = tuple(sorted(kw.items()))
    if key not in _NC_CACHE:
        _NC_CACHE[key] = _build(**kw)
    return _NC_CACHE[key]


def _prep_inputs(x_b, theta_w, phi_w, g_w, out_w, bn_gamma, bn_beta,
                 fc1_w, fc1_b, fc2_w, fc2_b, nn=N):
    bf = ml_dtypes.bfloat16
    return {
        "x": np.ascontiguousarray(x_b, dtype=np.float32),
        "wq_t": np.ascontiguousarray(np.asarray(theta_w, np.float32).T).astype(bf),
        "wk_t": np.ascontiguousarray(np.asarray(phi_w, np.float32).T).astype(bf),
        "wg_t": np.ascontiguousarray(np.asarray(g_w, np.float32).T).astype(bf),
        "wo_t": np.ascontiguousarray(np.asarray(out_w, np.float32).T).astype(bf),
        "fc1_wt": np.ascontiguousarray(
            (np.asarray(fc1_w, np.float32) / nn).T).astype(np.float32),
        "fc1_b": np.ascontiguousarray(fc1_b, dtype=np.float32),
        "fc2_wt": np.ascontiguousarray(
            np.asarray(fc2_w, np.float32).T).astype(np.float32),
        "fc2_bn": np.ascontiguousarray(-np.asarray(fc2_b, np.float32)),
        "bn_gamma": np.ascontiguousarray(bn_gamma, dtype=np.float32),
        "bn_beta": np.ascontiguousarray(bn_beta, dtype=np.float32),
    }


def _run(inputs, trace=False):
    nc = _get_nc()
    x = np.asarray(inputs["x"], dtype=np.float32)
    xs = x.reshape(B, C, N)
    in_maps = [
        _prep_inputs(xs[i], inputs["theta_w"], inputs["phi_w"], inputs["g_w"],
                     inputs["out_w"], inputs["bn_gamma"], inputs["bn_beta"],
                     inputs["fc1_w"], inputs["fc1_b"], inputs["fc2_w"],
                     inputs["fc2_b"])
        for i in range(B)
    ]
    res = run_bass_kernel_spmd(nc, in_maps, list(range(B)), trace=trace)
    out = np.stack([np.asarray(res.results[i]["out"], dtype=np.float32)
                    for i in range(B)])
    return out.reshape(B, C, H, W), res


def kernel(**inputs) -> np.ndarray:
    out, _ = _run(inputs, trace=False)
    return out


# revision 10
# speedup vs baseline: 1.1841x; 1.1841x over previous
"""NonLocalBlock (nn_NonLocalBlock_80221399155245) — Trainium2 Bass kernel.

Sharding: data-parallel over batch B=8, one batch item per NeuronCore.
Per-core pipeline (xf = x[b] as [C=256, N=4096]):
  theta = Wq @ xf, phi = Wk @ xf        [I=128, N]  (bf16, I-major)
  gT    = (Wg @ xf).T                   (N-major 128-chunks, bf16)
  Main loop over nb=512 column blocks, software-pipelined 3 deep so the
  PE never idles (p-state ramp) while ACT exp paces the loop:
    seg kb: PE: logits(kb) interleaved with outT(kb-1), bc(kb-1),
                ypj(kb-2)
            ACT: exp(kb) groups (PSUM->SBUF bf16)
            DVE: pairwise tree colsum(kb-1) in bf16, ypj drain(kb-3)
                 + bn_stats, on(kb-1) = outT/bc divide
  BatchNorm batch-stats are AllReduced across the 8 cores (sync-BN
  exact), SE channel attention computed in the prologue, residual fused
  in the epilogue via ACT affine (scale/bias per channel) + DVE add.
"""

import numpy as np
import ml_dtypes
import concourse.bass as bass
import concourse.tile as tile
from concourse import bacc, mybir
from concourse.bass_utils import run_bass_kernel_spmd

F32 = mybir.dt.float32
BF16 = mybir.dt.bfloat16
F32R = mybir.dt.float32r
AF = mybir.ActivationFunctionType
ALU = mybir.AluOpType

C = 256     # channels
I = 128     # inter channels
R = 64      # SE reduction
P = 128     # SBUF partitions
B = 8       # batch == cores
H = W = 64
N = H * W   # 4096 pixels
NB = 512    # n-block columns
CHUNK_GROUP = 2   # logits chunks per exp-activation group


def _build(n_cores=B, nn=N, nb=NB, chunk_group=CHUNK_GROUP, total_pixels=None):
    M = nn // P
    NBLK = nn // nb
    GRP = M // chunk_group
    assert M % chunk_group == 0
    if total_pixels is None:
        total_pixels = n_cores * nn
    sm_scale = float(1.0 / np.sqrt(np.float32(I)))

    nc = bacc.Bacc("TRN2", target_bir_lowering=False, debug=False,
                   num_devices=n_cores)

    x_d = nc.declare_dram_parameter("x", [C, nn], F32, isOutput=False)
    wq_d = nc.declare_dram_parameter("wq_t", [C, I], BF16, isOutput=False)
    wk_d = nc.declare_dram_parameter("wk_t", [C, I], BF16, isOutput=False)
    wg_d = nc.declare_dram_parameter("wg_t", [C, I], BF16, isOutput=False)
    wo_d = nc.declare_dram_parameter("wo_t", [I, C], BF16, isOutput=False)
    fc1w_d = nc.declare_dram_parameter("fc1_wt", [C, R], F32, isOutput=False)
    fc1b_d = nc.declare_dram_parameter("fc1_b", [R], F32, isOutput=False)
    fc2w_d = nc.declare_dram_parameter("fc2_wt", [R, C], F32, isOutput=False)
    fc2bn_d = nc.declare_dram_parameter("fc2_bn", [C], F32, isOutput=False)
    gam_d = nc.declare_dram_parameter("bn_gamma", [C], F32, isOutput=False)
    bet_d = nc.declare_dram_parameter("bn_beta", [C], F32, isOutput=False)
    out_d = nc.declare_dram_parameter("out", [C, nn], F32, isOutput=True)

    bn_in = nc.dram_tensor("bn_in", [P, 4], F32)
    bn_out = nc.dram_tensor("bn_out", [P, 4], F32,
                            addr_space="Shared" if n_cores > 4 else "Local")

    with tile.TileContext(nc) as tc:
        import contextlib
        with contextlib.ExitStack() as stack:
            sing = stack.enter_context(tc.tile_pool(name="sing", bufs=1))

            xf32 = [sing.tile([P, nn], F32, tag=f"xf32_{cc}", name=f"xf32_{cc}")
                    for cc in range(2)]
            xbf = [sing.tile([P, nn], BF16, tag=f"xbf_{cc}", name=f"xbf_{cc}")
                   for cc in range(2)]
            theta = sing.tile([P, nn], BF16, tag="theta", name="theta")
            phi = sing.tile([P, nn], BF16, tag="phi", name="phi")
            gT = sing.tile([P, nn], BF16, tag="gT", name="gT")
            ysb_all = sing.tile([P, 2, nn], BF16, tag="ysb_all", name="ysb_all")
            bnst = [sing.tile([P, NBLK, 6], F32, tag=f"bnst_{cc}",
                              name=f"bnst_{cc}") for cc in range(2)]

            wq = sing.tile([P, 2, I], BF16, tag="wq", name="wq")
            wk = sing.tile([P, 2, I], BF16, tag="wk", name="wk")
            wg = sing.tile([P, 2, I], BF16, tag="wg", name="wg")
            wo = sing.tile([P, 2, P], BF16, tag="wo", name="wo")
            fc1w = sing.tile([P, 2, R], F32, tag="fc1w", name="fc1w")
            fc1b = sing.tile([R, 1], F32, tag="fc1b", name="fc1b")
            fc2w = sing.tile([R, 2, P], F32, tag="fc2w", name="fc2w")
            fc2bn = sing.tile([P, 2], F32, tag="fc2bn", name="fc2bn")
            gam = sing.tile([P, 2], F32, tag="gam", name="gam")
            bet = sing.tile([P, 2], F32, tag="bet", name="bet")
            ones128 = sing.tile([P, P], BF16, tag="ones128", name="ones128")
            chw = sing.tile([P, 2], F32, tag="chw", name="chw")
            pooled = sing.tile([P, 2], F32, tag="pooled", name="pooled")

            nc.vector.memset(ones128, 1.0)

            # weights first (tiny), then x in column-chunk pairs
            nc.sync.dma_start(out=wq, in_=wq_d.rearrange("(a p) i -> p a i", p=P))
            nc.sync.dma_start(out=wk, in_=wk_d.rearrange("(a p) i -> p a i", p=P))
            nc.sync.dma_start(out=wg, in_=wg_d.rearrange("(a p) i -> p a i", p=P))
            nc.sync.dma_start(out=wo, in_=wo_d.rearrange("i (a c) -> i a c", a=2))
            nc.sync.dma_start(out=fc1w, in_=fc1w_d.rearrange("(a p) r -> p a r", p=P))
            nc.sync.dma_start(out=fc1b, in_=fc1b_d[:, None])
            nc.sync.dma_start(out=fc2w, in_=fc2w_d.rearrange("r (a c) -> r a c", a=2))
            nc.sync.dma_start(out=fc2bn, in_=fc2bn_d.rearrange("(a p) -> p a", p=P))
            nc.sync.dma_start(out=gam, in_=gam_d.rearrange("(a p) -> p a", p=P))
            nc.sync.dma_start(out=bet, in_=bet_d.rearrange("(a p) -> p a", p=P))

            NXCH = 4            # x DMA column chunks
            xw = nn // NXCH
            for t in range(NXCH):
                sl = slice(t * xw, (t + 1) * xw)
                for cc in range(2):
                    nc.sync.dma_start(out=xf32[cc][:, sl],
                                      in_=x_d[cc * P:(cc + 1) * P, sl])
                for cc in range(2):
                    nc.vector.tensor_copy(xbf[cc][:, sl], xf32[cc][:, sl])

            # ---- prologue: QKV projections + SE ----
            NCH = nn // 512
            with tc.tile_pool(name="proj_ps", bufs=4, space="PSUM") as pps, \
                 tc.tile_pool(name="g_ps", bufs=2, space="PSUM") as gps, \
                 tc.tile_pool(name="se_ps", bufs=1, space="PSUM") as seps:
                # phi fully first (block 0 logits need all of phi), theta
                # chunk 0 early, then the rest
                proj_order = [(wk, phi, t) for t in range(NCH)]
                proj_order.insert(1, (wq, theta, 0))
                proj_order += [(wq, theta, t) for t in range(1, NCH)]
                for (wt, dst, t) in proj_order:
                    ps = pps.tile([P, 512], F32, tag="proj", name="proj")
                    for cc in range(2):
                        nc.tensor.matmul(
                            ps[:], wt[:, cc, :],
                            xbf[cc][:, t * 512:(t + 1) * 512],
                            start=(cc == 0), stop=(cc == 1))
                    nc.scalar.copy(dst[:, t * 512:(t + 1) * 512], ps[:])

                for q in range(M // 4):
                    ps2 = gps.tile([P, 4, I], F32, tag="gproj", name="gproj")
                    for j in range(4):
                        mj = q * 4 + j
                        for cc in range(2):
                            nc.tensor.matmul(
                                ps2[:, j, :], xbf[cc][:, mj * P:(mj + 1) * P],
                                wg[:, cc, :], start=(cc == 0), stop=(cc == 1))
                    nc.vector.tensor_copy(
                        gT[:, q * 4 * I:(q + 1) * 4 * I],
                        ps2[:, :, :].rearrange("p a b -> p (a b)"))

                # SE channel attention (prologue: PSUM free, loads Exp table
                # before the main loop needs it)
                for cc in range(2):
                    nc.vector.reduce_sum(pooled[:, cc:cc + 1], xf32[cc][:],
                                         axis=mybir.AxisListType.X)
                hps = seps.tile([R, 1], F32, tag="se", name="se_h")
                for cc in range(2):
                    nc.tensor.matmul(hps[:], fc1w[:, cc, :], pooled[:, cc:cc + 1],
                                     start=(cc == 0), stop=(cc == 1))
                hsb = sing.tile([R, 1], F32, tag="hsb", name="hsb")
                nc.scalar.activation(hsb[:], hps[:], AF.Relu, bias=fc1b[:])
                for cc in range(2):
                    zps = seps.tile([P, 1], F32, tag="se2", name="se_z")
                    nc.tensor.matmul(zps[:], fc2w[:, cc, :], hsb[:],
                                     start=True, stop=True)
                    esb = sing.tile([P, 1], F32, tag=f"esb_{cc}", name=f"esb_{cc}")
                    nc.scalar.activation(esb[:], zps[:], AF.Exp,
                                         bias=fc2bn[:, cc:cc + 1], scale=-1.0)
                    nc.vector.tensor_scalar_add(esb[:], esb[:], 1.0)
                    nc.vector.reciprocal(chw[:, cc:cc + 1], esb[:])

            # ---- main attention loop: software-pipelined 3 deep ----
            with tc.tile_pool(name="lg", bufs=2, space="PSUM") as lg, \
                 tc.tile_pool(name="aux_ps", bufs=1, space="PSUM") as aux, \
                 tc.tile_pool(name="pTp", bufs=2) as pTp, \
                 tc.tile_pool(name="tree", bufs=1) as tree, \
                 tc.tile_pool(name="smalls", bufs=2) as smalls:

                outT_ps = aux.tile([P, nb], F32, tag="outT", name="outT")
                bc_ps = aux.tile([P, nb], F32, tag="bc", name="bc")
                ypj_ps = aux.tile([P, 2, nb], F32, tag="ypj", name="ypj")

                t1 = tree.tile([P, M * nb // 2], BF16, tag="t1", name="t1")
                t2 = tree.tile([P, max(M * nb // 4, nb)], BF16,
                               tag="t2", name="t2")

                pTs = [None] * NBLK
                sums = [None] * NBLK
                ons = [None] * NBLK
                # outT(kb-1) is front-loaded into the first half of the
                # groups so its PSUM is complete (and normalized) mid-
                # segment, clearing the WAR for outT(kb) at the next
                # segment's start.
                FL = 2 * chunk_group   # outT chunks per group when front-
                FGR = M // FL          # ...loading: done after FGR groups

                for kb in range(NBLK + 3):
                    # --- DVE: ypj drain + bn_stats for block kb-3 ---
                    if 3 <= kb <= NBLK + 2:
                        k3 = kb - 3
                        nc.vector.tensor_scalar(
                            ysb_all[:, :, k3 * nb:(k3 + 1) * nb],
                            ypj_ps[:, :, :], 1.0, None, ALU.mult)
                        for cc in range(2):
                            nc.vector.bn_stats(
                                out=bnst[cc][:, k3, :],
                                in_=ysb_all[:, cc, k3 * nb:(k3 + 1) * nb])

                    # --- DVE: colsum tree for block kb-1 (bf16 pairwise) ---
                    if 1 <= kb <= NBLK:
                        pT_p = pTs[kb - 1]
                        sm = smalls.tile([P, nb], BF16, tag="sums", name="sums")
                        sums[kb - 1] = sm
                        cur, w = pT_p, M * nb
                        lvl = 0
                        while w > 2 * nb:
                            dst = t1 if lvl % 2 == 0 else t2
                            nc.vector.tensor_tensor(
                                dst[:, :w // 2], cur[:, :w // 2],
                                cur[:, w // 2:w], ALU.add)
                            cur, w, lvl = dst, w // 2, lvl + 1
                        nc.vector.tensor_tensor(
                            sm[:], cur[:, :nb], cur[:, nb:2 * nb], ALU.add)

                    def emit_outT(kk, mj):
                        nc.tensor.matmul(
                            outT_ps[:], gT[:, mj * I:(mj + 1) * I],
                            pTs[kk][:, mj * nb:(mj + 1) * nb],
                            start=(mj == 0), stop=(mj == M - 1),
                            skip_group_check=True)

                    def emit_bc_norm(kk):
                        # PE: broadcast colsums; DVE: inv + on = outT*inv
                        nc.tensor.matmul(bc_ps[:], ones128[:], sums[kk][:],
                                         start=True, stop=True)
                        inv_sb = smalls.tile([P, nb], F32, tag="inv",
                                             name="inv")
                        nc.vector.reciprocal_approx_fast(out=inv_sb[:],
                                                         in_=bc_ps[:])
                        on_sb = smalls.tile([P, nb], BF16, tag="on", name="on")
                        ons[kk] = on_sb
                        nc.vector.tensor_tensor(on_sb[:], outT_ps[:],
                                                inv_sb[:], ALU.mult)

                    # --- PE: logits(kb) with outT(kb-1) front-loaded ---
                    if kb < NBLK:
                        th_sl = theta[:, kb * nb:(kb + 1) * nb]
                        pT = pTp.tile([P, M * nb], BF16, tag="pT", name="pT")
                        pTs[kb] = pT
                        for g in range(GRP):
                            lgt = lg.tile([P, chunk_group, nb], F32,
                                          tag="lg", name="lg")
                            for j in range(chunk_group):
                                mj = g * chunk_group + j
                                nc.tensor.matmul(
                                    lgt[:, j, :], phi[:, mj * P:(mj + 1) * P],
                                    th_sl, start=True, stop=True)
                            nc.scalar.activation(
                                pT[:, g * chunk_group * nb:
                                   (g + 1) * chunk_group * nb],
                                lgt[:, :, :].rearrange("p a b -> p (a b)"),
                                AF.Exp, scale=sm_scale)
                            if kb >= 1 and g < FGR:
                                for j in range(FL):
                                    emit_outT(kb - 1, g * FL + j)
                            if kb >= 1 and g == FGR + 1:
                                emit_bc_norm(kb - 1)
                    elif kb == NBLK:
                        for mj in range(M):
                            emit_outT(kb - 1, mj)
                        emit_bc_norm(kb - 1)

                    # --- PE: ypj(kb-2) = Wo @ on ---
                    if 2 <= kb <= NBLK + 1:
                        for cc in range(2):
                            nc.tensor.matmul(ypj_ps[:, cc, :], wo[:, cc, :],
                                             ons[kb - 2][:],
                                             start=True, stop=True)

            # ---- epilogue: sync-BN, affine, residual ----
            with tc.tile_pool(name="epi", bufs=2) as epi:
                stats = sing.tile([P, 4], F32, tag="stats", name="stats")
                for cc in range(2):
                    mv = epi.tile([P, 2], F32, tag="mv", name="mv")
                    nc.vector.bn_aggr(out=mv[:], in_=bnst[cc][:, :, :])
                    # sum = mean*nn ; sumsq = (var + mean^2)*nn
                    nc.vector.tensor_scalar_mul(stats[:, cc:cc + 1],
                                                mv[:, 0:1], float(nn))
                    m2 = epi.tile([P, 1], F32, tag="m2", name="m2")
                    nc.vector.tensor_tensor(m2[:], mv[:, 0:1], mv[:, 0:1],
                                            ALU.mult)
                    nc.vector.tensor_tensor(m2[:], mv[:, 1:2], m2[:], ALU.add)
                    nc.vector.tensor_scalar_mul(stats[:, 2 + cc:3 + cc],
                                                m2[:], float(nn))
                nc.sync.dma_start(out=bn_in[:], in_=stats[:])
                nc.gpsimd.collective_compute(
                    "AllReduce", ALU.add,
                    replica_groups=[list(range(n_cores))],
                    ins=[bn_in[:]], outs=[bn_out[:]])
                stats_g = sing.tile([P, 4], F32, tag="stats_g", name="stats_g")
                nc.gpsimd.dma_start(out=stats_g[:], in_=bn_out[:])

                inv_np = 1.0 / float(total_pixels)
                half = nn // 2
                for cc in range(2):
                    mean = epi.tile([P, 1], F32, tag="mean", name="mean")
                    ex2 = epi.tile([P, 1], F32, tag="ex2", name="ex2")
                    nc.vector.tensor_scalar_mul(mean[:], stats_g[:, cc:cc + 1],
                                                inv_np)
                    nc.vector.tensor_scalar_mul(ex2[:], stats_g[:, 2 + cc:3 + cc],
                                                inv_np)
                    var = epi.tile([P, 1], F32, tag="var", name="var")
                    nc.vector.tensor_tensor(var[:], mean[:], mean[:], ALU.mult)
                    nc.vector.tensor_tensor(var[:], ex2[:], var[:], ALU.subtract)
                    nc.vector.tensor_scalar_add(var[:], var[:], 1e-5)
                    rv = epi.tile([P, 1], F32, tag="rv", name="rv")
                    nc.vector.reciprocal(rv[:], var[:])
                    istd = epi.tile([P, 1], F32, tag="istd", name="istd")
                    nc.scalar.activation(istd[:], rv[:], AF.Sqrt)
                    g1 = epi.tile([P, 1], F32, tag="g1", name="g1")
                    nc.vector.tensor_tensor(g1[:], istd[:], gam[:, cc:cc + 1],
                                            ALU.mult)
                    A = epi.tile([P, 1], F32, tag="A", name="A")
                    nc.vector.tensor_tensor(A[:], g1[:], chw[:, cc:cc + 1],
                                            ALU.mult)
                    Bt = epi.tile([P, 1], F32, tag="Bt", name="Bt")
                    nc.vector.tensor_tensor(Bt[:], mean[:], g1[:], ALU.mult)
                    nc.vector.tensor_tensor(Bt[:], bet[:, cc:cc + 1], Bt[:],
                                            ALU.subtract)
                    nc.vector.tensor_tensor(Bt[:], Bt[:], chw[:, cc:cc + 1],
                                            ALU.mult)

                    for h in range(4):
                        qw = nn // 4
                        sl = slice(h * qw, (h + 1) * qw)
                        tf = epi.tile([P, qw], F32, tag="tf", name="tf")
                        nc.scalar.activation(tf[:], ysb_all[:, cc, sl],
                                             AF.Identity, bias=Bt[:],
                                             scale=A[:])
                        osb = epi.tile([P, qw], F32, tag="osb", name="osb")
                        nc.vector.tensor_tensor(osb[:], tf[:],
                                                xf32[cc][:, sl], ALU.add)
                        nc.sync.dma_start(out=out_d[cc * P:(cc + 1) * P, sl],
                                          in_=osb[:])

    nc.compile()
    return nc


_NC_CACHE = {}


def _get_nc(**kw):
    key = tuple(sorted(kw.items()))
    if key not in _NC_CACHE:
        _NC_CACHE[key] = _build(**kw)
    return _NC_CACHE[key]


def _prep_inputs(x_b, theta_w, phi_w, g_w, out_w, bn_gamma, bn_beta,
                 fc1_w, fc1_b, fc2_w, fc2_b, nn=N):
    bf = ml_dtypes.bfloat16
    return {
        "x": np.ascontiguousarray(x_b, dtype=np.float32),
        "wq_t": np.ascontiguousarray(np.asarray(theta_w, np.float32).T).astype(bf),
        "wk_t": np.ascontiguousarray(np.asarray(phi_w, np.float32).T).astype(bf),
        "wg_t": np.ascontiguousarray(np.asarray(g_w, np.float32).T).astype(bf),
        "wo_t": np.ascontiguousarray(np.asarray(out_w, np.float32).T).astype(bf),
        "fc1_wt": np.ascontiguousarray(
            (np.asarray(fc1_w, np.float32) / nn).T).astype(np.float32),
        "fc1_b": np.ascontiguousarray(fc1_b, dtype=np.float32),
        "fc2_wt": np.ascontiguousarray(
            np.asarray(fc2_w, np.float32).T).astype(np.float32),
        "fc2_bn": np.ascontiguousarray(-np.asarray(fc2_b, np.float32)),
        "bn_gamma": np.ascontiguousarray(bn_gamma, dtype=np.float32),
        "bn_beta": np.ascontiguousarray(bn_beta, dtype=np.float32),
    }


def _run(inputs, trace=False):
    nc = _get_nc()
    x = np.asarray(inputs["x"], dtype=np.float32)
    xs = x.reshape(B, C, N)
    in_maps = [
        _prep_inputs(xs[i], inputs["theta_w"], inputs["phi_w"], inputs["g_w"],
                     inputs["out_w"], inputs["bn_gamma"], inputs["bn_beta"],
                     inputs["fc1_w"], inputs["fc1_b"], inputs["fc2_w"],
                     inputs["fc2_b"])
        for i in range(B)
    ]
    res = run_bass_kernel_spmd(nc, in_maps, list(range(B)), trace=trace)
    out = np.stack([np.asarray(res.results[i]["out"], dtype=np.float32)
                    for i in range(B)])
    return out.reshape(B, C, H, W), res


def kernel(**inputs) -> np.ndarray:
    out, _ = _run(inputs, trace=False)
    return out
